# revision 7
# baseline (speedup 1.0000x reference)
"""Trainium2 Bass kernel for nn_Conduits_8899172237900 (glacier conduit GNN step).

Self-contained: takes the FULL inputs (reference.py names), shards across the
8 NeuronCores internally, runs one SPMD Bass/Tile NEFF, and returns the FULL
[2, N_NODES] float32 output (stack of [S_new, flux_div]).

Sharding strategy (the hint's halo replication taken to per-reference
granularity): nodes are sharded 250k/core; at shard time every link quantity
that a node's K=6 link references need (conduit size and hydraulic head at
the link's head/tail nodes, the link's Reynolds number and length) is
replicated to the reference site, so on-device all gathers become purely
sequential streams. Each core then:
  - computes the link physics (transmissivity -> discharge Q, dissipation D)
    at each of its 6*250k reference sites,
  - reduces over K=6 (signed flux divergence, mean dissipation),
  - runs the per-node melt / creep-closure RK4 conduit-size update.
All device memory access is contiguous streaming (memory-roofline bound);
no cross-core exchange is needed at runtime because the halo data is
replicated into each core's shard.

This formulation avoids the per-element indirect-DMA gather because on this
walrus build InstDMACopy with a dynamic AP lowers to a per-PARTITION block
gather (one descriptor per partition using idx[p, 0] only) — verified on
hardware — which cannot express 12M scalar gathers at any usable rate.
"""
import contextlib
import ctypes
import sys
import types

for _p in ("/opt/trn_rl_repo",):
    if _p not in sys.path:
        sys.path.append(_p)

import numpy as np

import concourse.bass as bass
import concourse.tile as tile
from concourse import mybir

F32 = mybir.dt.float32
I32 = mybir.dt.int32
OP = mybir.AluOpType

# physical constants (Glacier defaults)
G = 9.81
RHO_I = 917.0
RHO_W = 1000.0
NU = 1.787e-6
OMEGA = 1e-3
L_HEAT = 334000.0
A_ICE = 6e-24
DT = 0.01

N_CORES = 8
P = 128
TN = 128                      # nodes per partition per tile
N_NODES = 2_000_000
NN = N_NODES // N_CORES       # 250_000 nodes per core
NN_PAD = 262_144              # 16 tiles of 128*128 nodes


# ---------------------------------------------------------------------------
# Environment shims
# ---------------------------------------------------------------------------
def _install_axon_ntff_hook():
    """bass_utils under axon imports antenv.axon_hooks unconditionally when
    tracing; this image lacks the module. Recreate it from the
    libaxon_pjrt.so C ABI so profiling works when a caller enables it."""
    try:
        import antenv.axon_hooks  # noqa: F401
        return
    except ImportError:
        pass
    try:
        import antenv
    except ImportError:
        return

    so_path = "/opt/axon/libaxon_pjrt.so"

    def _make_hook():
        try:
            lib = ctypes.CDLL(so_path)
        except OSError:
            return None
        if not hasattr(lib, "axon_start_nrt_profile"):
            return None
        lib.axon_start_nrt_profile.argtypes = [
            ctypes.POINTER(ctypes.c_int64),
            ctypes.c_size_t,
        ]
        lib.axon_start_nrt_profile.restype = ctypes.c_int64
        lib.axon_stop_nrt_profile.argtypes = [ctypes.c_char_p]
        lib.axon_stop_nrt_profile.restype = ctypes.c_int64

        @contextlib.contextmanager
        def _hook(output_dir, device_ids):
            import jax

            jax.devices()
            if device_ids:
                ids = (ctypes.c_int64 * len(device_ids))(*device_ids)
                rc = lib.axon_start_nrt_profile(ids, len(device_ids))
            else:
                rc = lib.axon_start_nrt_profile(None, 0)
            if rc != 0:
                raise RuntimeError(f"axon_start_nrt_profile rc={rc}")
            try:
                yield
            finally:
                n = lib.axon_stop_nrt_profile(str(output_dir).encode())
                print(f"profile: {n} file(s) in {output_dir}", file=sys.stderr)

        return _hook

    mod = types.ModuleType("antenv.axon_hooks")
    state = {"hook": _make_hook()}
    mod.set_axon_ntff_profile_hook = lambda h: state.__setitem__("hook", h)
    mod.get_axon_ntff_profile_hook = lambda: state["hook"]
    sys.modules["antenv.axon_hooks"] = mod
    import antenv

    antenv.axon_hooks = mod


def _install_drain_patch():
    """This walrus build rejects >1 sync-wait per CTRL op; split the Tile
    kernel-tail drain's waits across preceding SP nops."""
    if getattr(tile.TileContext, "_drain_patch_installed", False):
        return

    MAXW = 1

    def _drain_and_barrier(self, tick_clock, wait_clock):
        nc = self.nc
        probe = nc.sync.nop(nofuse=True)
        wait_clock.add_sem_waits(
            probe.ins, tile.ScopedClock({None: tick_clock.global_clock})
        )
        waits = list(probe.ins.sync_info.on_wait) if probe.ins.sync_info else []
        if len(waits) > MAXW:
            probe.ins.sync_info.on_wait = waits[:MAXW]
            rest = waits[MAXW:]
            while rest:
                extra = nc.sync.nop(nofuse=True)
                chunk, rest = rest[:MAXW], rest[MAXW:]
                if extra.ins.sync_info is None:
                    extra.ins.sync_info = mybir.SyncInfo(on_wait=chunk, on_update=[])
                else:
                    extra.ins.sync_info.on_wait = chunk
        nc.sync.drain()

        nc.all_engine_barrier()
        assert self.sems is not None
        popped = nc._tile_sem_poison_stack.pop()
        assert popped is self._sem_poison
        nc.clear_and_free_semaphores(list(self.sems.allocated().values()))
        nc.all_engine_barrier()

    tile.TileContext._drain_and_barrier = _drain_and_barrier
    tile.TileContext._drain_patch_installed = True


def _legalize_waits(nc, max_waits=1):
    """Move excess per-instruction sem-waits onto same-engine NoOps inserted
    immediately before the instruction (this walrus rejects multi-wait ops)."""
    for f in nc.m.functions:
        for bb in f.blocks:
            insts = bb.instructions
            new_list = []
            changed = False
            for ins in insts:
                si = ins.sync_info
                if si is not None and len(si.on_wait) > max_waits:
                    waits = list(si.on_wait)
                    extra = waits[max_waits:]
                    while extra:
                        chunk, extra = extra[:max_waits], extra[max_waits:]
                        nop = mybir.InstNoOp(
                            name=f"waitsplit-{nc.next_id()}",
                            sync_info=mybir.SyncInfo(on_wait=chunk, on_update=[]),
                            bass_nofuse=True,
                            engine=ins.engine,
                        )
                        nc.register_instruction(nop)
                        new_list.append(nop)
                        changed = True
                    si.on_wait = waits[:max_waits]
                new_list.append(ins)
            if changed:
                insts[:] = new_list


# ---------------------------------------------------------------------------
# Kernel builder
# ---------------------------------------------------------------------------
def _build():
    n_ntiles = NN_PAD // (P * TN)
    TN6 = TN * 6

    nc = bass.Bass(num_devices=N_CORES)

    def par(name, shape):
        return nc.declare_dram_parameter(name, shape, F32, isOutput=False)

    csh = par("csh", [NN_PAD * 6])    # conduit_size[head[l]] per reference
    cst = par("cst", [NN_PAD * 6])    # conduit_size[tail[l]]
    hhd = par("hhd", [NN_PAD * 6])    # hydraulic_head[head[l]]
    htl = par("htl", [NN_PAD * 6])    # hydraulic_head[tail[l]]
    reyr = par("reyr", [NN_PAD * 6])  # reynolds[l]
    lenr = par("lenr", [NN_PAD * 6])  # length_of_link[l]
    dirs = par("dirs", [NN_PAD * 6])  # link_dirs as f32
    area = par("area", [NN_PAD])
    ice = par("ice", [NN_PAD])
    bed = par("bed", [NN_PAD])
    geo = par("geo", [NN_PAD])
    s0 = par("s0", [NN_PAD])
    hh = par("hh", [NN_PAD])
    out = nc.declare_dram_parameter("out", [2, NN_PAD], F32, isOutput=True)

    C_TRANS = 0.125 * G / (12.0 * NU)     # (0.5)^3 folded into G/(12 nu)
    C_DISS = RHO_W * G / 6.0
    C_MELT = (1.0 / RHO_W - 1.0 / RHO_I) / L_HEAT

    def v6(x):
        return x.rearrange("(t p f) -> t p f", p=P, f=TN6)

    def v1(x):
        return x.rearrange("(t p f) -> t p f", p=P, f=TN)

    csh_v, cst_v, hhd_v, htl_v = v6(csh), v6(cst), v6(hhd), v6(htl)
    reyr_v, lenr_v, dirs_v = v6(reyr), v6(lenr), v6(dirs)
    area_v, ice_v, bed_v, geo_v, s0_v, hh_v = (
        v1(area), v1(ice), v1(bed), v1(geo), v1(s0), v1(hh))
    out0_v = out[0, :].rearrange("(t p f) -> t p f", p=P, f=TN)
    out1_v = out[1, :].rearrange("(t p f) -> t p f", p=P, f=TN)

    with tile.TileContext(nc) as tc:
        with (
            tc.tile_pool(name="io", bufs=3) as io,
            tc.tile_pool(name="tmp", bufs=2) as tmp,
        ):
            for t in range(n_ntiles):
                def load6(view, tag):
                    tl = io.tile([P, TN6], F32, tag=tag)
                    nc.sync.dma_start(tl[:], view[t])
                    return tl

                def load1(view, tag):
                    tl = io.tile([P, TN], F32, tag=tag)
                    nc.sync.dma_start(tl[:], view[t])
                    return tl

                csh_t = load6(csh_v, "csh")
                cst_t = load6(cst_v, "cst")
                hhd_t = load6(hhd_v, "hhd")
                htl_t = load6(htl_v, "htl")
                rey_t = load6(reyr_v, "rey")
                len_t = load6(lenr_v, "len")
                dirs_t = load6(dirs_v, "dirs")
                area_t = load1(area_v, "area")
                ice_t = load1(ice_v, "ice")
                bed_t = load1(bed_v, "bed")
                geo_t = load1(geo_v, "geo")
                s0_t = load1(s0_v, "s0")
                hh_t = load1(hh_v, "hh")

                # ---- link physics at each reference site ----
                size_s = tmp.tile([P, TN6], F32, tag="size_s")
                nc.gpsimd.tensor_add(size_s[:], csh_t[:], cst_t[:])
                dh = tmp.tile([P, TN6], F32, tag="dh")
                nc.gpsimd.tensor_sub(dh[:], hhd_t[:], htl_t[:])
                rlen = tmp.tile([P, TN6], F32, tag="rlen")
                nc.vector.reciprocal(rlen[:], len_t[:])
                grad = tmp.tile([P, TN6], F32, tag="grad")
                nc.vector.tensor_mul(grad[:], dh[:], rlen[:])
                den = tmp.tile([P, TN6], F32, tag="den")
                nc.scalar.activation(
                    den[:], rey_t[:], mybir.ActivationFunctionType.Copy,
                    bias=1.0, scale=OMEGA,
                )
                rden = tmp.tile([P, TN6], F32, tag="rden")
                nc.vector.reciprocal(rden[:], den[:])
                s2 = tmp.tile([P, TN6], F32, tag="s2")
                nc.scalar.activation(
                    s2[:], size_s[:], mybir.ActivationFunctionType.Square,
                )
                s3 = tmp.tile([P, TN6], F32, tag="s3")
                nc.vector.tensor_mul(s3[:], s2[:], size_s[:])
                tq = tmp.tile([P, TN6], F32, tag="tq")
                nc.vector.tensor_mul(tq[:], s3[:], rden[:])
                q = tmp.tile([P, TN6], F32, tag="q")
                nc.vector.scalar_tensor_tensor(
                    q[:], tq[:], C_TRANS, grad[:], OP.mult, OP.mult
                )
                d = tmp.tile([P, TN6], F32, tag="d")
                nc.vector.tensor_mul(d[:], q[:], grad[:])

                # ---- per-node reductions over K=6 ----
                prod = tmp.tile([P, TN6], F32, tag="prod")
                nc.vector.tensor_mul(prod[:], q[:], dirs_t[:])
                flux = tmp.tile([P, TN], F32, tag="flux")
                nc.vector.reduce_sum(
                    flux[:], prod[:].rearrange("p (n k) -> p n k", k=6),
                    axis=mybir.AxisListType.X,
                )
                diss = tmp.tile([P, TN], F32, tag="diss")
                nc.vector.reduce_sum(
                    diss[:], d[:].rearrange("p (n k) -> p n k", k=6),
                    axis=mybir.AxisListType.X,
                    apply_absolute_value=True,
                )

                rarea = tmp.tile([P, TN], F32, tag="rarea")
                nc.vector.reciprocal(rarea[:], area_t[:])
                fdiv = io.tile([P, TN], F32, tag="fdiv")
                nc.vector.tensor_mul(fdiv[:], flux[:], rarea[:])
                nc.sync.dma_start(out1_v[t], fdiv[:])

                # ---- melt / creep closure / RK4 ----
                melt = tmp.tile([P, TN], F32, tag="melt")
                nc.vector.scalar_tensor_tensor(
                    melt[:], diss[:], C_DISS, geo_t[:], OP.mult, OP.add
                )
                nc.vector.tensor_scalar(melt[:], melt[:], C_MELT, None, OP.mult)

                hb = tmp.tile([P, TN], F32, tag="hb")
                nc.vector.tensor_sub(hb[:], hh_t[:], bed_t[:])
                t1 = tmp.tile([P, TN], F32, tag="t1")
                nc.vector.tensor_scalar(t1[:], ice_t[:], RHO_I * G, None, OP.mult)
                neff = tmp.tile([P, TN], F32, tag="neff")
                nc.vector.scalar_tensor_tensor(
                    neff[:], hb[:], -(RHO_W * G), t1[:], OP.mult, OP.add
                )
                n2 = tmp.tile([P, TN], F32, tag="n2")
                nc.vector.tensor_mul(n2[:], neff[:], neff[:])
                creep = tmp.tile([P, TN], F32, tag="creep")
                nc.vector.tensor_mul(creep[:], n2[:], neff[:])
                nc.vector.tensor_scalar(creep[:], creep[:], A_ICE, None, OP.mult)

                tk = tmp.tile([P, TN], F32, tag="tk")
                k1 = tmp.tile([P, TN], F32, tag="k1")
                nc.vector.tensor_mul(tk[:], creep[:], s0_t[:])
                nc.vector.tensor_sub(k1[:], melt[:], tk[:])
                sst = tmp.tile([P, TN], F32, tag="sst")
                nc.vector.scalar_tensor_tensor(
                    sst[:], k1[:], DT / 2, s0_t[:], OP.mult, OP.add
                )
                k2 = tmp.tile([P, TN], F32, tag="k2")
                nc.vector.tensor_mul(tk[:], creep[:], sst[:])
                nc.vector.tensor_sub(k2[:], melt[:], tk[:])
                nc.vector.scalar_tensor_tensor(
                    sst[:], k2[:], DT / 2, s0_t[:], OP.mult, OP.add
                )
                k3 = tmp.tile([P, TN], F32, tag="k3")
                nc.vector.tensor_mul(tk[:], creep[:], sst[:])
                nc.vector.tensor_sub(k3[:], melt[:], tk[:])
                nc.vector.scalar_tensor_tensor(
                    sst[:], k3[:], DT, s0_t[:], OP.mult, OP.add
                )
                k4 = tmp.tile([P, TN], F32, tag="k4")
                nc.vector.tensor_mul(tk[:], creep[:], sst[:])
                nc.vector.tensor_sub(k4[:], melt[:], tk[:])

                u = tmp.tile([P, TN], F32, tag="u")
                nc.vector.tensor_add(u[:], k1[:], k4[:])
                v = tmp.tile([P, TN], F32, tag="v")
                nc.vector.tensor_add(v[:], k2[:], k3[:])
                nc.vector.scalar_tensor_tensor(u[:], v[:], 2.0, u[:], OP.mult, OP.add)
                snew = io.tile([P, TN], F32, tag="snew")
                nc.vector.scalar_tensor_tensor(
                    snew[:], u[:], DT / 6, s0_t[:], OP.mult, OP.add
                )
                nc.sync.dma_start(out0_v[t], snew[:])

    _legalize_waits(nc, max_waits=1)
    return nc


_NC_CACHE = None


def _get_nc():
    global _NC_CACHE
    if _NC_CACHE is None:
        _install_axon_ntff_hook()
        _install_drain_patch()
        _NC_CACHE = _build()
    return _NC_CACHE


# ---------------------------------------------------------------------------
# Host-side shard / unshard
# ---------------------------------------------------------------------------
def _shard_inputs(inputs):
    cs = np.asarray(inputs["conduit_size"], np.float32)
    h = np.asarray(inputs["hydraulic_head"], np.float32)
    reynolds = np.asarray(inputs["reynolds"], np.float32)
    ice = np.asarray(inputs["ice_thickness"], np.float32)
    bed = np.asarray(inputs["bedrock_elevation"], np.float32)
    geo = np.asarray(inputs["geothermal_heat_flux"], np.float32)
    length = np.asarray(inputs["length_of_link"], np.float32)
    area = np.asarray(inputs["area_at_node"], np.float32)
    headi = np.asarray(inputs["node_at_link_head"], np.int64)
    taili = np.asarray(inputs["node_at_link_tail"], np.int64)
    lan = np.asarray(inputs["links_at_node"], np.int64)
    dirs = np.asarray(inputs["link_dirs_at_node"], np.int32)

    lf = lan.reshape(-1)              # link id per (node, slot) reference
    hf = headi[lf]                    # that link's head node
    tf = taili[lf]
    ref = {
        "csh": cs[hf], "cst": cs[tf], "hhd": h[hf], "htl": h[tf],
        "reyr": reynolds[lf], "lenr": length[lf],
        "dirs": dirs.reshape(-1).astype(np.float32),
    }

    in_maps = []
    for c in range(N_CORES):
        ns, ne = c * NN, (c + 1) * NN
        m = {}
        for k, vv in ref.items():
            o = (np.ones if k == "lenr" else np.zeros)(NN_PAD * 6, np.float32)
            o[: NN * 6] = vv[ns * 6: ne * 6]
            m[k] = o

        def padn(src, fill=1.0):
            o = np.full(NN_PAD, fill, np.float32)
            o[:NN] = src[ns:ne]
            return o

        m.update(
            area=padn(area), ice=padn(ice), bed=padn(bed),
            geo=padn(geo, 0.0), s0=padn(cs), hh=padn(h, 0.0),
        )
        in_maps.append(m)
    return in_maps


def _run(inputs, trace=False, trace_cores=None):
    from concourse.bass_utils import run_bass_kernel_spmd

    nc = _get_nc()
    in_maps = _shard_inputs(inputs)
    res = run_bass_kernel_spmd(
        nc, in_maps, list(range(N_CORES)), trace=trace, trace_cores=trace_cores
    )
    parts = [res.results[c]["out"][:, :NN] for c in range(N_CORES)]
    return np.concatenate(parts, axis=1), res


def kernel(**inputs):
    out, _ = _run(inputs)
    return out


# revision 8
# speedup vs baseline: 1.0156x; 1.0156x over previous
"""Trainium2 Bass kernel for nn_Conduits_8899172237900 (glacier conduit GNN step).

Self-contained: takes the FULL inputs (reference.py names), shards across the
8 NeuronCores internally, runs one SPMD Bass/Tile NEFF, and returns the FULL
[2, N_NODES] float32 output (stack of [S_new, flux_div]).

Sharding strategy (the hint's halo replication taken to per-reference
granularity): nodes are sharded 250k/core; at shard time every link quantity
that a node's K=6 link references need (conduit size and hydraulic head at
the link's head/tail nodes, the link's Reynolds number and length) is
replicated to the reference site, so on-device all gathers become purely
sequential streams. Each core then:
  - computes the link physics (transmissivity -> discharge Q, dissipation D)
    at each of its 6*250k reference sites,
  - reduces over K=6 (signed flux divergence, mean dissipation),
  - runs the per-node melt / creep-closure RK4 conduit-size update.
All device memory access is contiguous streaming (memory-roofline bound);
no cross-core exchange is needed at runtime because the halo data is
replicated into each core's shard.

This formulation avoids the per-element indirect-DMA gather because on this
walrus build InstDMACopy with a dynamic AP lowers to a per-PARTITION block
gather (one descriptor per partition using idx[p, 0] only) — verified on
hardware — which cannot express 12M scalar gathers at any usable rate.
"""
import contextlib
import ctypes
import sys
import types

for _p in ("/opt/trn_rl_repo",):
    if _p not in sys.path:
        sys.path.append(_p)

import numpy as np

import concourse.bass as bass
import concourse.tile as tile
from concourse import mybir

F32 = mybir.dt.float32
I32 = mybir.dt.int32
OP = mybir.AluOpType

# physical constants (Glacier defaults)
G = 9.81
RHO_I = 917.0
RHO_W = 1000.0
NU = 1.787e-6
OMEGA = 1e-3
L_HEAT = 334000.0
A_ICE = 6e-24
DT = 0.01

N_CORES = 8
P = 128
TN = 128                      # nodes per partition per tile
N_NODES = 2_000_000
NN = N_NODES // N_CORES       # 250_000 nodes per core
NN_PAD = 262_144              # 16 tiles of 128*128 nodes


# ---------------------------------------------------------------------------
# Environment shims
# ---------------------------------------------------------------------------
def _install_axon_ntff_hook():
    """bass_utils under axon imports antenv.axon_hooks unconditionally when
    tracing; this image lacks the module. Recreate it from the
    libaxon_pjrt.so C ABI so profiling works when a caller enables it."""
    try:
        import antenv.axon_hooks  # noqa: F401
        return
    except ImportError:
        pass
    try:
        import antenv
    except ImportError:
        return

    so_path = "/opt/axon/libaxon_pjrt.so"

    def _make_hook():
        try:
            lib = ctypes.CDLL(so_path)
        except OSError:
            return None
        if not hasattr(lib, "axon_start_nrt_profile"):
            return None
        lib.axon_start_nrt_profile.argtypes = [
            ctypes.POINTER(ctypes.c_int64),
            ctypes.c_size_t,
        ]
        lib.axon_start_nrt_profile.restype = ctypes.c_int64
        lib.axon_stop_nrt_profile.argtypes = [ctypes.c_char_p]
        lib.axon_stop_nrt_profile.restype = ctypes.c_int64

        @contextlib.contextmanager
        def _hook(output_dir, device_ids):
            import jax

            jax.devices()
            if device_ids:
                ids = (ctypes.c_int64 * len(device_ids))(*device_ids)
                rc = lib.axon_start_nrt_profile(ids, len(device_ids))
            else:
                rc = lib.axon_start_nrt_profile(None, 0)
            if rc != 0:
                raise RuntimeError(f"axon_start_nrt_profile rc={rc}")
            try:
                yield
            finally:
                n = lib.axon_stop_nrt_profile(str(output_dir).encode())
                print(f"profile: {n} file(s) in {output_dir}", file=sys.stderr)

        return _hook

    mod = types.ModuleType("antenv.axon_hooks")
    state = {"hook": _make_hook()}
    mod.set_axon_ntff_profile_hook = lambda h: state.__setitem__("hook", h)
    mod.get_axon_ntff_profile_hook = lambda: state["hook"]
    sys.modules["antenv.axon_hooks"] = mod
    import antenv

    antenv.axon_hooks = mod


def _install_drain_patch():
    """This walrus build rejects >1 sync-wait per CTRL op; split the Tile
    kernel-tail drain's waits across preceding SP nops."""
    if getattr(tile.TileContext, "_drain_patch_installed", False):
        return

    MAXW = 1

    def _drain_and_barrier(self, tick_clock, wait_clock):
        nc = self.nc
        probe = nc.sync.nop(nofuse=True)
        wait_clock.add_sem_waits(
            probe.ins, tile.ScopedClock({None: tick_clock.global_clock})
        )
        waits = list(probe.ins.sync_info.on_wait) if probe.ins.sync_info else []
        if len(waits) > MAXW:
            probe.ins.sync_info.on_wait = waits[:MAXW]
            rest = waits[MAXW:]
            while rest:
                extra = nc.sync.nop(nofuse=True)
                chunk, rest = rest[:MAXW], rest[MAXW:]
                if extra.ins.sync_info is None:
                    extra.ins.sync_info = mybir.SyncInfo(on_wait=chunk, on_update=[])
                else:
                    extra.ins.sync_info.on_wait = chunk
        nc.sync.drain()

        nc.all_engine_barrier()
        assert self.sems is not None
        popped = nc._tile_sem_poison_stack.pop()
        assert popped is self._sem_poison
        nc.clear_and_free_semaphores(list(self.sems.allocated().values()))
        nc.all_engine_barrier()

    tile.TileContext._drain_and_barrier = _drain_and_barrier
    tile.TileContext._drain_patch_installed = True


def _legalize_waits(nc, max_waits=1):
    """Move excess per-instruction sem-waits onto same-engine NoOps inserted
    immediately before the instruction (this walrus rejects multi-wait ops)."""
    for f in nc.m.functions:
        for bb in f.blocks:
            insts = bb.instructions
            new_list = []
            changed = False
            for ins in insts:
                si = ins.sync_info
                if si is not None and len(si.on_wait) > max_waits:
                    waits = list(si.on_wait)
                    extra = waits[max_waits:]
                    while extra:
                        chunk, extra = extra[:max_waits], extra[max_waits:]
                        nop = mybir.InstNoOp(
                            name=f"waitsplit-{nc.next_id()}",
                            sync_info=mybir.SyncInfo(on_wait=chunk, on_update=[]),
                            bass_nofuse=True,
                            engine=ins.engine,
                        )
                        nc.register_instruction(nop)
                        new_list.append(nop)
                        changed = True
                    si.on_wait = waits[:max_waits]
                new_list.append(ins)
            if changed:
                insts[:] = new_list


# ---------------------------------------------------------------------------
# Kernel builder
# ---------------------------------------------------------------------------
def _build():
    n_ntiles = NN_PAD // (P * TN)
    TN6 = TN * 6

    nc = bass.Bass(num_devices=N_CORES)

    def par(name, shape):
        return nc.declare_dram_parameter(name, shape, F32, isOutput=False)

    csh = par("csh", [NN_PAD * 6])    # conduit_size[head[l]] per reference
    cst = par("cst", [NN_PAD * 6])    # conduit_size[tail[l]]
    hhd = par("hhd", [NN_PAD * 6])    # hydraulic_head[head[l]]
    htl = par("htl", [NN_PAD * 6])    # hydraulic_head[tail[l]]
    reyr = par("reyr", [NN_PAD * 6])  # reynolds[l]
    lenr = par("lenr", [NN_PAD * 6])  # length_of_link[l]
    dirs = par("dirs", [NN_PAD * 6])  # link_dirs as f32
    area = par("area", [NN_PAD])
    ice = par("ice", [NN_PAD])
    bed = par("bed", [NN_PAD])
    geo = par("geo", [NN_PAD])
    s0 = par("s0", [NN_PAD])
    hh = par("hh", [NN_PAD])
    out = nc.declare_dram_parameter("out", [2, NN_PAD], F32, isOutput=True)

    C_TRANS = 0.125 * G / (12.0 * NU)     # (0.5)^3 folded into G/(12 nu)
    C_DISS = RHO_W * G / 6.0
    C_MELT = (1.0 / RHO_W - 1.0 / RHO_I) / L_HEAT

    def v6(x):
        return x.rearrange("(t p f) -> t p f", p=P, f=TN6)

    def v1(x):
        return x.rearrange("(t p f) -> t p f", p=P, f=TN)

    csh_v, cst_v, hhd_v, htl_v = v6(csh), v6(cst), v6(hhd), v6(htl)
    reyr_v, lenr_v, dirs_v = v6(reyr), v6(lenr), v6(dirs)
    area_v, ice_v, bed_v, geo_v, s0_v, hh_v = (
        v1(area), v1(ice), v1(bed), v1(geo), v1(s0), v1(hh))
    out0_v = out[0, :].rearrange("(t p f) -> t p f", p=P, f=TN)
    out1_v = out[1, :].rearrange("(t p f) -> t p f", p=P, f=TN)

    with tile.TileContext(nc) as tc:
        with (
            tc.tile_pool(name="io", bufs=3) as io,
            tc.tile_pool(name="tmp", bufs=2) as tmp,
        ):
            for t in range(n_ntiles):
                def load6(view, tag):
                    tl = io.tile([P, TN6], F32, tag=tag)
                    nc.sync.dma_start(tl[:], view[t])
                    return tl

                def load1(view, tag):
                    tl = io.tile([P, TN], F32, tag=tag)
                    nc.sync.dma_start(tl[:], view[t])
                    return tl

                csh_t = load6(csh_v, "csh")
                cst_t = load6(cst_v, "cst")
                hhd_t = load6(hhd_v, "hhd")
                htl_t = load6(htl_v, "htl")
                rey_t = load6(reyr_v, "rey")
                len_t = load6(lenr_v, "len")
                dirs_t = load6(dirs_v, "dirs")
                area_t = load1(area_v, "area")
                ice_t = load1(ice_v, "ice")
                bed_t = load1(bed_v, "bed")
                geo_t = load1(geo_v, "geo")
                s0_t = load1(s0_v, "s0")
                hh_t = load1(hh_v, "hh")

                # ---- link physics at each reference site ----
                size_s = tmp.tile([P, TN6], F32, tag="size_s")
                nc.vector.tensor_add(size_s[:], csh_t[:], cst_t[:])
                dh = tmp.tile([P, TN6], F32, tag="dh")
                nc.vector.tensor_sub(dh[:], hhd_t[:], htl_t[:])
                rlen = tmp.tile([P, TN6], F32, tag="rlen")
                nc.vector.reciprocal(rlen[:], len_t[:])
                grad = tmp.tile([P, TN6], F32, tag="grad")
                nc.vector.tensor_mul(grad[:], dh[:], rlen[:])
                den = tmp.tile([P, TN6], F32, tag="den")
                nc.scalar.activation(
                    den[:], rey_t[:], mybir.ActivationFunctionType.Copy,
                    bias=1.0, scale=OMEGA,
                )
                rden = tmp.tile([P, TN6], F32, tag="rden")
                nc.vector.reciprocal(rden[:], den[:])
                s2 = tmp.tile([P, TN6], F32, tag="s2")
                nc.scalar.activation(
                    s2[:], size_s[:], mybir.ActivationFunctionType.Square,
                )
                s3 = tmp.tile([P, TN6], F32, tag="s3")
                nc.vector.tensor_mul(s3[:], s2[:], size_s[:])
                tq = tmp.tile([P, TN6], F32, tag="tq")
                nc.vector.tensor_mul(tq[:], s3[:], rden[:])
                q = tmp.tile([P, TN6], F32, tag="q")
                nc.vector.scalar_tensor_tensor(
                    q[:], tq[:], C_TRANS, grad[:], OP.mult, OP.mult
                )
                d = tmp.tile([P, TN6], F32, tag="d")
                nc.vector.tensor_mul(d[:], q[:], grad[:])

                # ---- per-node reductions over K=6 ----
                prod = tmp.tile([P, TN6], F32, tag="prod")
                nc.vector.tensor_mul(prod[:], q[:], dirs_t[:])
                flux = tmp.tile([P, TN], F32, tag="flux")
                nc.vector.reduce_sum(
                    flux[:], prod[:].rearrange("p (n k) -> p n k", k=6),
                    axis=mybir.AxisListType.X,
                )
                diss = tmp.tile([P, TN], F32, tag="diss")
                nc.vector.reduce_sum(
                    diss[:], d[:].rearrange("p (n k) -> p n k", k=6),
                    axis=mybir.AxisListType.X,
                    apply_absolute_value=True,
                )

                rarea = tmp.tile([P, TN], F32, tag="rarea")
                nc.vector.reciprocal(rarea[:], area_t[:])
                fdiv = io.tile([P, TN], F32, tag="fdiv")
                nc.vector.tensor_mul(fdiv[:], flux[:], rarea[:])
                nc.sync.dma_start(out1_v[t], fdiv[:])

                # ---- melt / creep closure / RK4 ----
                melt = tmp.tile([P, TN], F32, tag="melt")
                nc.vector.scalar_tensor_tensor(
                    melt[:], diss[:], C_DISS, geo_t[:], OP.mult, OP.add
                )
                nc.vector.tensor_scalar(melt[:], melt[:], C_MELT, None, OP.mult)

                hb = tmp.tile([P, TN], F32, tag="hb")
                nc.vector.tensor_sub(hb[:], hh_t[:], bed_t[:])
                t1 = tmp.tile([P, TN], F32, tag="t1")
                nc.vector.tensor_scalar(t1[:], ice_t[:], RHO_I * G, None, OP.mult)
                neff = tmp.tile([P, TN], F32, tag="neff")
                nc.vector.scalar_tensor_tensor(
                    neff[:], hb[:], -(RHO_W * G), t1[:], OP.mult, OP.add
                )
                n2 = tmp.tile([P, TN], F32, tag="n2")
                nc.vector.tensor_mul(n2[:], neff[:], neff[:])
                creep = tmp.tile([P, TN], F32, tag="creep")
                nc.vector.tensor_mul(creep[:], n2[:], neff[:])
                nc.vector.tensor_scalar(creep[:], creep[:], A_ICE, None, OP.mult)

                tk = tmp.tile([P, TN], F32, tag="tk")
                k1 = tmp.tile([P, TN], F32, tag="k1")
                nc.vector.tensor_mul(tk[:], creep[:], s0_t[:])
                nc.vector.tensor_sub(k1[:], melt[:], tk[:])
                sst = tmp.tile([P, TN], F32, tag="sst")
                nc.vector.scalar_tensor_tensor(
                    sst[:], k1[:], DT / 2, s0_t[:], OP.mult, OP.add
                )
                k2 = tmp.tile([P, TN], F32, tag="k2")
                nc.vector.tensor_mul(tk[:], creep[:], sst[:])
                nc.vector.tensor_sub(k2[:], melt[:], tk[:])
                nc.vector.scalar_tensor_tensor(
                    sst[:], k2[:], DT / 2, s0_t[:], OP.mult, OP.add
                )
                k3 = tmp.tile([P, TN], F32, tag="k3")
                nc.vector.tensor_mul(tk[:], creep[:], sst[:])
                nc.vector.tensor_sub(k3[:], melt[:], tk[:])
                nc.vector.scalar_tensor_tensor(
                    sst[:], k3[:], DT, s0_t[:], OP.mult, OP.add
                )
                k4 = tmp.tile([P, TN], F32, tag="k4")
                nc.vector.tensor_mul(tk[:], creep[:], sst[:])
                nc.vector.tensor_sub(k4[:], melt[:], tk[:])

                u = tmp.tile([P, TN], F32, tag="u")
                nc.vector.tensor_add(u[:], k1[:], k4[:])
                v = tmp.tile([P, TN], F32, tag="v")
                nc.vector.tensor_add(v[:], k2[:], k3[:])
                nc.vector.scalar_tensor_tensor(u[:], v[:], 2.0, u[:], OP.mult, OP.add)
                snew = io.tile([P, TN], F32, tag="snew")
                nc.vector.scalar_tensor_tensor(
                    snew[:], u[:], DT / 6, s0_t[:], OP.mult, OP.add
                )
                nc.sync.dma_start(out0_v[t], snew[:])

    _legalize_waits(nc, max_waits=1)
    return nc


_NC_CACHE = None


def _get_nc():
    global _NC_CACHE
    if _NC_CACHE is None:
        _install_axon_ntff_hook()
        _install_drain_patch()
        _NC_CACHE = _build()
    return _NC_CACHE


# ---------------------------------------------------------------------------
# Host-side shard / unshard
# ---------------------------------------------------------------------------
def _shard_inputs(inputs):
    cs = np.asarray(inputs["conduit_size"], np.float32)
    h = np.asarray(inputs["hydraulic_head"], np.float32)
    reynolds = np.asarray(inputs["reynolds"], np.float32)
    ice = np.asarray(inputs["ice_thickness"], np.float32)
    bed = np.asarray(inputs["bedrock_elevation"], np.float32)
    geo = np.asarray(inputs["geothermal_heat_flux"], np.float32)
    length = np.asarray(inputs["length_of_link"], np.float32)
    area = np.asarray(inputs["area_at_node"], np.float32)
    headi = np.asarray(inputs["node_at_link_head"], np.int64)
    taili = np.asarray(inputs["node_at_link_tail"], np.int64)
    lan = np.asarray(inputs["links_at_node"], np.int64)
    dirs = np.asarray(inputs["link_dirs_at_node"], np.int32)

    lf = lan.reshape(-1)              # link id per (node, slot) reference
    hf = headi[lf]                    # that link's head node
    tf = taili[lf]
    ref = {
        "csh": cs[hf], "cst": cs[tf], "hhd": h[hf], "htl": h[tf],
        "reyr": reynolds[lf], "lenr": length[lf],
        "dirs": dirs.reshape(-1).astype(np.float32),
    }

    in_maps = []
    for c in range(N_CORES):
        ns, ne = c * NN, (c + 1) * NN
        m = {}
        for k, vv in ref.items():
            o = (np.ones if k == "lenr" else np.zeros)(NN_PAD * 6, np.float32)
            o[: NN * 6] = vv[ns * 6: ne * 6]
            m[k] = o

        def padn(src, fill=1.0):
            o = np.full(NN_PAD, fill, np.float32)
            o[:NN] = src[ns:ne]
            return o

        m.update(
            area=padn(area), ice=padn(ice), bed=padn(bed),
            geo=padn(geo, 0.0), s0=padn(cs), hh=padn(h, 0.0),
        )
        in_maps.append(m)
    return in_maps


def _run(inputs, trace=False, trace_cores=None):
    from concourse.bass_utils import run_bass_kernel_spmd

    nc = _get_nc()
    in_maps = _shard_inputs(inputs)
    res = run_bass_kernel_spmd(
        nc, in_maps, list(range(N_CORES)), trace=trace, trace_cores=trace_cores
    )
    parts = [res.results[c]["out"][:, :NN] for c in range(N_CORES)]
    return np.concatenate(parts, axis=1), res


def kernel(**inputs):
    out, _ = _run(inputs)
    return out
